# revision 3
# baseline (speedup 1.0000x reference)
"""Trainium2 Bass kernel for nn_ContinuousPool.

Computes, for x:(32,96,128,128) f32 and pool_strength:(1,96,1,1) f32:
    cur = x
    repeat 10: cur = cur + s * (maxpool3x3_same(cur) - cur)
    out = avgpool2x2(cur)            -> (32,96,64,64)

Strategy (v2):
  - Pure data parallel over 8 cores: 384 images/core as 3 chunks of 128
    (one image per SBUF partition).
  - fp16 on device (host casts x): DVE tensor_tensor runs in 2x_1p mode.
  - Rescaled recurrence  u' = u + (s/(1-s)) * maxpool3x3(u)  so each step
    is 4 tensor_max + 1 scale + 1 add; the scale (w = c*v) runs on the
    Activation engine, off the DVE critical path.
  - All DVE ops are half-frame (N=8192): measured per-op cost is
    sub-linear in N on this silicon (4.49us @8192 vs 10.4us @16384).
  - Chunks 0 and 1 run interleaved (double-buffered u/r/v) so ACT work
    and DVE inter-op gaps overlap; chunk 2 follows solo.
  - w = c*v overlays r's interior (disjoint lifetimes within a step).
  - Final avgpool2x2 in fp16; output scaled on ACT and cast fp16->f32 by
    a gpsimd (SWDGE) casting DMA on the way to DRAM.
"""

import sys

if "/opt/trn_rl_repo" not in sys.path:
    sys.path.insert(0, "/opt/trn_rl_repo")

import numpy as np

B, C, H, W = 32, 96, 128, 128
T = 10
N_CORES = 8
B_PER_CORE = B // N_CORES          # 4
IMGS = B_PER_CORE * C              # 384 images per core
CHUNK = 128                        # images (partitions) per chunk
NCHUNK = IMGS // CHUNK             # 3

UOFF = 2                           # u interior col0 offset (even -> 4B aligned)
USTR = 130                         # u row stride (pad col each side)
USZ = UOFF + USTR * H + 2
RSTR = 128
RSZ = RSTR * (H + 2)               # pad row above and below
VSZ = H * W                        # 16384
HB = H // 2                        # rows per half

_CACHE = {}


def _build(R=None):
    import concourse.bacc as bacc
    import concourse.mybir as mybir
    from concourse import tile

    f16 = mybir.dt.float16
    f32 = mybir.dt.float32

    nc = bacc.Bacc("TRN2", target_bir_lowering=False, debug=False,
                   num_devices=N_CORES)

    x_d = nc.dram_tensor("x", [IMGS, H * W], f16, kind="ExternalInput")
    c_d = nc.dram_tensor("cvec", [IMGS, 1], f32, kind="ExternalInput")
    f_d = nc.dram_tensor("fvec", [IMGS, 1], f32, kind="ExternalInput")
    y_d = nc.dram_tensor("y", [IMGS, (H // 2) * (W // 2)], f32,
                         kind="ExternalOutput")

    with tile.TileContext(nc, num_cores=N_CORES) as tc:
        with tc.tile_pool(name="main", bufs=1) as pool:
            u_t = [pool.tile([128, USZ], f16, name=f"u{i}", tag=f"u{i}")
                   for i in range(2)]
            r_t = [pool.tile([128, RSZ], f16, name=f"r{i}", tag=f"r{i}")
                   for i in range(2)]
            v_t = [pool.tile([128, VSZ], f16, name=f"v{i}", tag=f"v{i}")
                   for i in range(2)]
            cs_t = pool.tile([128, 2 * NCHUNK], f32, tag="cs")

            NEGINF = float("-inf")
            for i in range(2):
                nc.vector.memset(u_t[i][:, :], NEGINF)
                nc.vector.memset(r_t[i][:, :], NEGINF)

            def u_view(c, h, dx=0):
                t = u_t[c % 2]
                base = UOFF + dx + USTR * HB * h
                return t[:, base:base + USTR * HB].rearrange(
                    "p (h w) -> p h w", h=HB, w=USTR)[:, :, 0:W]

            def r_view(c, h, dy=0):
                t = r_t[c % 2]
                base = RSTR * (1 + dy) + RSTR * HB * h
                return t[:, base:base + RSTR * HB].rearrange(
                    "p (h w) -> p h w", h=HB, w=RSTR)

            def v_view(c, h):
                t = v_t[c % 2]
                return t[:, RSTR * HB * h:RSTR * HB * (h + 1)].rearrange(
                    "p (h w) -> p h w", h=HB, w=RSTR)

            def load_chunk(c):
                rows = slice(c * CHUNK, (c + 1) * CHUNK)
                x_v = x_d[rows, :].rearrange("p (h w) -> p h w", h=H, w=W)
                for h in (0, 1):
                    nc.sync.dma_start(u_view(c, h),
                                      x_v[:, HB * h:HB * (h + 1), :])

            def emit_steps(chunks, t):
                for opi in range(4):
                    for c in chunks:
                        for h in (0, 1):
                            if opi == 0:
                                nc.vector.tensor_max(
                                    r_view(c, h), u_view(c, h, -1),
                                    u_view(c, h, +1))
                            elif opi == 1:
                                nc.vector.tensor_max(
                                    r_view(c, h), r_view(c, h), u_view(c, h))
                            elif opi == 2:
                                nc.vector.tensor_max(
                                    v_view(c, h), r_view(c, h, -1),
                                    r_view(c, h, +1))
                            else:
                                nc.vector.tensor_max(
                                    v_view(c, h), v_view(c, h), r_view(c, h))
                for c in chunks:
                    for h in (0, 1):
                        nc.scalar.mul(r_view(c, h), v_view(c, h),
                                      cs_t[:, 2 * (c % NCHUNK):
                                           2 * (c % NCHUNK) + 1])
                for c in chunks:
                    for h in (0, 1):
                        nc.vector.tensor_add(
                            u_view(c, h), u_view(c, h), r_view(c, h))

            def emit_tail(c):
                ut = u_t[c % 2]
                vt = v_t[c % 2]
                rows = slice(c * CHUNK, (c + 1) * CHUNK)
                u4 = ut[:, UOFF:UOFF + USTR * H].rearrange(
                    "p (h w2 two) -> p h w2 two", h=H, w2=USTR // 2, two=2)
                ac = vt[:, 0:H * 64].rearrange("p (h w) -> p h w", h=H, w=64)
                for h in (0, 1):
                    rs = slice(HB * h, HB * (h + 1))
                    nc.vector.tensor_add(ac[:, rs], u4[:, rs, 0:64, 0:1],
                                         u4[:, rs, 0:64, 1:2])
                a3 = vt[:, 0:H * 64].rearrange(
                    "p (h2 two w) -> p h2 two w", h2=H // 2, two=2, w=64)
                bsum = vt[:, H * 64:H * 64 + 4096].rearrange(
                    "p (h w) -> p h w", h=64, w=64)
                nc.vector.tensor_add(bsum, a3[:, :, 0:1, :], a3[:, :, 1:2, :])
                bf = vt[:, H * 64 + 4096:H * 64 + 8192]
                nc.scalar.mul(bf, vt[:, H * 64:H * 64 + 4096],
                              cs_t[:, 2 * (c % NCHUNK) + 1:
                                   2 * (c % NCHUNK) + 2])
                nc.gpsimd.dma_start(
                    y_d[rows, :], bf.rearrange("p (a b) -> p a b", a=64, b=64))

            def body():
                for k in range(NCHUNK):
                    rows = slice(k * CHUNK, (k + 1) * CHUNK)
                    nc.sync.dma_start(cs_t[:, 2 * k:2 * k + 1], c_d[rows, :])
                    nc.sync.dma_start(cs_t[:, 2 * k + 1:2 * k + 2],
                                      f_d[rows, :])
                load_chunk(0)
                load_chunk(1)
                for t in range(T):
                    emit_steps((0, 1), t)
                emit_tail(0)
                load_chunk(2)
                emit_tail(1)
                for t in range(T):
                    emit_steps((2,), t)
                emit_tail(2)

            if R is None:
                body()
            else:
                with tc.For_i(0, R) as _i:
                    body()

    nc.compile()
    return nc


def _get_program():
    if "nc" not in _CACHE:
        _CACHE["nc"] = _build()
    return _CACHE["nc"]


def kernel(x: np.ndarray, pool_strength: np.ndarray) -> np.ndarray:
    from concourse.bass_utils import run_bass_kernel_spmd

    nc = _get_program()

    x16 = np.asarray(x, dtype=np.float16)
    s = np.asarray(pool_strength, dtype=np.float64).reshape(C)
    c_ch = (s / (1.0 - s)).astype(np.float32)                  # [C]
    f_ch = (((1.0 - s) ** T) * 0.25).astype(np.float32)        # [C]
    cvec = np.ascontiguousarray(np.tile(c_ch, B_PER_CORE)[:, None])  # [384,1]
    fvec = np.ascontiguousarray(np.tile(f_ch, B_PER_CORE)[:, None])

    in_maps = []
    for j in range(N_CORES):
        xj = np.ascontiguousarray(
            x16[j * B_PER_CORE:(j + 1) * B_PER_CORE].reshape(IMGS, H * W))
        in_maps.append({"x": xj, "cvec": cvec, "fvec": fvec})

    res = run_bass_kernel_spmd(nc, in_maps, list(range(N_CORES)))

    out = np.empty((B, C, H // 2, W // 2), dtype=np.float32)
    for j in range(N_CORES):
        yj = res.results[j]["y"].reshape(B_PER_CORE, C, H // 2, W // 2)
        out[j * B_PER_CORE:(j + 1) * B_PER_CORE] = yj
    return out
